# revision 1
# baseline (speedup 1.0000x reference)
"""Trainium2 Bass kernel for nn_GAT_Vanilla (2-layer GAT + BN/ELU + MLP head).

Strategy (8 NeuronCores, graph/data parallel):
- Nodes are bin-packed into 8 cores x 98 blocks x 128 slots (edge-balanced
  bins; a node permutation). Each core owns its slots' rows end to end.
- 3 SPMD launches: A) per-node matmuls producing h1/s1/d1/x_p for owned
  slots; B) layer-1 edge phase (gather h1[src] via dma_gather, segment
  softmax + weighted segment-sum via one-hot matmuls, BN+ELU, then the
  layer-2 node matmul h2/s2/d2); C) layer-2 edge phase + residual + MLP
  head + log_softmax.
- Between launches the host performs the halo exchange: it routes each
  core's h rows into per-(core,phase) compacted gather tables (int16 index
  space) and expands per-edge s_src/d_dst arrays. Pure indexing, no math.
- Edge bookkeeping (one-hot dst, tile/block structure, padding) is
  precomputed on the host from edge_index.

Self-contained: only needs numpy + the concourse/bass stack.
"""

import math
import numpy as np

import concourse.bass as bass
import concourse.bacc as bacc
import concourse.tile as tile
from concourse import mybir
from concourse.bass_utils import run_bass_kernel_spmd

F32 = mybir.dt.float32
I32 = mybir.dt.int32
I16 = mybir.dt.int16

# ---- problem constants (hardcoded per harness contract) ----
N, E, IN, HD, NH, OUT = 100000, 800000, 128, 32, 4, 40
D = HD * NH  # 128
EPS_BN = 1e-5
NEG = -60.0  # pad-edge logit -> exp == 0

# ---- tunables ----
TBL_DT = mybir.dt.bfloat16   # gather-table dtype (bfloat16 or float32)
NCORES = 8
NBLK = 98                    # node blocks per core (98*128 = 12544 slots)
PB = 20                      # blocks per gather phase (int16 index space)
GB = 4                       # blocks per dma_gather instruction
WCHUNK = 4                   # tiles per batched DMA in launch A

PROFILE = False              # set True (test.py) to collect exec times
LAST_EXEC_NS = []            # filled when PROFILE

_np_tbl_dt = None


def _np_dt():
    global _np_tbl_dt
    if _np_tbl_dt is None:
        if TBL_DT == mybir.dt.bfloat16:
            import ml_dtypes
            _np_tbl_dt = ml_dtypes.bfloat16
        else:
            _np_tbl_dt = np.float32
    return _np_tbl_dt


class Cfg:
    def __init__(self, n=N, e=E, ncores=NCORES, nblk=NBLK, pb=PB, gb=GB):
        self.n, self.e, self.ncores, self.nblk, self.pb, self.gb = \
            n, e, ncores, nblk, pb, gb
        self.slots = nblk * 128
        self.phases = []
        b = 0
        while b < nblk:
            self.phases.append(min(pb, nblk - b))
            b += pb
        self.T = None          # tiles per block (set by prep)
        self.phase_rows = None


# ----------------------------------------------------------------------------
# Host preprocessing: node binning, edge layout, phase tables
# ----------------------------------------------------------------------------

class Prep:
    pass


def host_prep(edge_index, cfg: Cfg):
    """Assign nodes to (core, block, pos) bins balancing per-block edge
    counts; lay out edges into (block, tile, lane) slots; build per-phase
    compacted int16 gather indexing."""
    import heapq
    n, e = cfg.n, cfg.e
    nbins = cfg.ncores * cfg.nblk
    src = np.concatenate([edge_index[0], np.arange(n)]).astype(np.int64)
    dst = np.concatenate([edge_index[1], np.arange(n)]).astype(np.int64)
    deg = np.bincount(dst, minlength=n)

    # greedy LPT: biggest-degree node to least-loaded bin with space
    order = np.argsort(-deg, kind="stable")
    heap = [(0, b) for b in range(nbins)]
    heapq.heapify(heap)
    bin_of = np.empty(n, np.int32)
    pos_of = np.empty(n, np.int32)
    bin_count = np.zeros(nbins, np.int32)
    spill = []
    for v in order:
        load, b = heapq.heappop(heap)
        bin_of[v] = b
        pos_of[v] = bin_count[b]
        bin_count[b] += 1
        if bin_count[b] < 128:
            heapq.heappush(heap, (load + deg[v], b))
        else:
            spill.append(b)
    # slot of node
    slot_of = bin_of.astype(np.int64) * 128 + pos_of
    core_of_bin = np.arange(nbins) // cfg.nblk

    # edges grouped by dst bin
    ebin = bin_of[dst]
    eorder = np.argsort(ebin, kind="stable")
    src_s, dst_s, ebin_s = src[eorder], dst[eorder], ebin[eorder]
    counts = np.bincount(ebin_s, minlength=nbins)
    starts = np.zeros(nbins + 1, np.int64)
    np.cumsum(counts, out=starts[1:])

    # dummy slots need one self edge each (weight-1, zero h contribution)
    n_dummy = np.maximum(0, 128 - bin_count)
    T = int(math.ceil((counts + n_dummy).max() / 128.0))
    cfg.T = T
    cap = T * 128

    per_core = []
    for c in range(cfg.ncores):
        pc = Prep()
        # linear edge arrays in (block, tile, lane) order
        src_slot = np.full(cfg.nblk * cap, -1, np.int64)   # -1 pad, -2 dummy
        dst_slot = np.full(cfg.nblk * cap, -1, np.int64)
        dst_local = np.zeros(cfg.nblk * cap, np.float32)
        for g in range(cfg.nblk):
            b = c * cfg.nblk + g
            s0, s1 = starts[b], starts[b + 1]
            cnt = s1 - s0
            base = g * cap
            src_slot[base:base + cnt] = slot_of[src_s[s0:s1]]
            dst_slot[base:base + cnt] = slot_of[dst_s[s0:s1]]
            dst_local[base:base + cnt] = pos_of[dst_s[s0:s1]]
            nd = n_dummy[b]
            if nd:
                src_slot[base + cnt:base + cnt + nd] = -2
                dst_slot[base + cnt:base + cnt + nd] = \
                    b * 128 + np.arange(bin_count[b], 128)
                dst_local[base + cnt:base + cnt + nd] = \
                    np.arange(bin_count[b], 128)
        pc.src_slot, pc.dst_slot = src_slot, dst_slot
        # [lane, q] layouts
        pc.dst_local = np.ascontiguousarray(
            dst_local.reshape(cfg.nblk * T, 128).T)

        # per-phase compacted index space
        pc.uniq = []
        pc.idx16 = []
        b0 = 0
        for pblocks in cfg.phases:
            lo, hi = b0 * cap, (b0 + pblocks) * cap
            ss = src_slot[lo:hi]
            real = ss >= 0
            u = np.unique(ss[real])
            lookup = np.zeros(cfg.ncores * cfg.slots, np.int16)
            lookup[u] = np.arange(1, len(u) + 1, dtype=np.int16)
            ids = np.zeros(hi - lo, np.int16)
            ids[real] = lookup[ss[real]]
            # wrapped int16 layout: element i -> [i % 16, i // 16], x8 replicate
            wrapped = np.tile(ids.reshape(-1, 16).T, (8, 1))
            pc.uniq.append(u)
            pc.idx16.append(np.ascontiguousarray(wrapped))
            b0 += pblocks
        per_core.append(pc)

    cfg.phase_rows = 1 + max(
        len(u) for pc in per_core for u in pc.uniq)
    prep = Prep()
    prep.per_core = per_core
    prep.slot_of = slot_of
    prep.cfg = cfg
    return prep


def expand_sd(prep, cfg, s_full, d_full):
    """Per-edge [s_src | d_dst] arrays, [128, nblk*T, 8] f32 per core."""
    out = []
    for pc in prep.per_core:
        sd = np.zeros((cfg.nblk * cfg.T * 128, 8), np.float32)
        real = pc.src_slot >= 0
        sd[real, 0:4] = s_full[pc.src_slot[real]]
        sd[real, 4:8] = d_full[pc.dst_slot[real]]
        sd[pc.src_slot == -1, 0:4] = NEG
        # dummy-self edges: s=d=0 -> weight 1 (already zeros)
        sd = sd.reshape(cfg.nblk * cfg.T, 128, 8).transpose(1, 0, 2)
        out.append(np.ascontiguousarray(sd))
    return out


def build_tables(prep, cfg, h_full):
    """Per-(core, phase) gather tables [phase_rows, 128] (row 0 zero)."""
    tabs = []
    for pc in prep.per_core:
        tl = []
        for u in pc.uniq:
            t = np.zeros((cfg.phase_rows, 128), h_full.dtype)
            t[1:1 + len(u)] = h_full[u]
            tl.append(t)
        tabs.append(tl)
    return tabs


# ----------------------------------------------------------------------------
# Device kernels
# ----------------------------------------------------------------------------

def build_launch_a(cfg: Cfg):
    nc = bacc.Bacc("TRN2", target_bir_lowering=False, debug=False,
                   num_devices=cfg.ncores)
    xT = nc.dram_tensor("xT", [128, cfg.slots], F32, kind="ExternalInput").ap()
    res_W = nc.dram_tensor("res_W", [128, 128], F32, kind="ExternalInput").ap()
    res_b_col = nc.dram_tensor("res_b_col", [128, 1], F32,
                               kind="ExternalInput").ap()
    res_b_rep = nc.dram_tensor("res_b_rep", [128, 128], F32,
                               kind="ExternalInput").ap()
    W1cat = nc.dram_tensor("W1cat", [128, 136], F32, kind="ExternalInput").ap()
    h1 = nc.dram_tensor("h1", [128, cfg.nblk, 128], TBL_DT,
                        kind="ExternalOutput").ap()
    sd1 = nc.dram_tensor("sd1", [128, cfg.nblk, 8], F32,
                         kind="ExternalOutput").ap()
    xp = nc.dram_tensor("xp", [128, cfg.nblk, 128], F32,
                        kind="ExternalOutput").ap()

    WC = WCHUNK
    nchunk = (cfg.nblk + WC - 1) // WC
    with tile.TileContext(nc) as tc:
        with (
            tc.tile_pool(name="const", bufs=1) as cp,
            tc.tile_pool(name="io", bufs=2) as iop,
            tc.tile_pool(name="work", bufs=2) as wp,
            tc.tile_pool(name="psa", bufs=2, space="PSUM") as psa,
            tc.tile_pool(name="psb", bufs=2, space="PSUM") as psb,
            tc.tile_pool(name="psh", bufs=2, space="PSUM") as psh,
        ):
            rw = cp.tile([128, 128], F32)
            nc.sync.dma_start(out=rw[:], in_=res_W)
            rbc = cp.tile([128, 1], F32)
            nc.sync.dma_start(out=rbc[:], in_=res_b_col)
            rbr = cp.tile([128, 128], F32)
            nc.sync.dma_start(out=rbr[:], in_=res_b_rep)
            w1 = cp.tile([128, 136], F32)
            nc.sync.dma_start(out=w1[:], in_=W1cat)

            for ch in range(nchunk):
                i0, i1 = ch * WC, min((ch + 1) * WC, cfg.nblk)
                nt = i1 - i0
                xt = iop.tile([128, WC, 128], F32, tag="xt")
                nc.sync.dma_start(
                    out=xt[:, 0:nt, :],
                    in_=xT[:, i0 * 128:i1 * 128].rearrange(
                        "p (t q) -> p t q", t=nt))
                xpc = iop.tile([128, WC, 128], F32, tag="xpc")
                h1c = iop.tile([128, WC, 128], TBL_DT, tag="h1c")
                sdc = iop.tile([128, WC, 8], F32, tag="sdc")
                for i in range(i0, i1):
                    t = i - i0
                    pa = psa.tile([128, 128], F32, tag="a")
                    nc.tensor.matmul(out=pa[:], lhsT=xt[:, t, :], rhs=rw[:],
                                     start=True, stop=True)
                    nc.vector.tensor_tensor(out=xpc[:, t, :], in0=pa[:],
                                            in1=rbr[:], op=mybir.AluOpType.add)
                    pb_ = psb.tile([128, 128], F32, tag="b")
                    nc.tensor.matmul(out=pb_[:], lhsT=rw[:], rhs=xt[:, t, :],
                                     start=True, stop=True)
                    xpT = wp.tile([128, 128], F32, tag="xpT")
                    nc.scalar.activation(
                        out=xpT[:], in_=pb_[:],
                        func=mybir.ActivationFunctionType.Identity,
                        bias=rbc[:])
                    ph = psh.tile([128, 136], F32, tag="h")
                    nc.tensor.matmul(out=ph[:], lhsT=xpT[:], rhs=w1[:],
                                     start=True, stop=True)
                    nc.vector.tensor_copy(h1c[:, t, :], ph[:, 0:128])
                    nc.vector.tensor_copy(sdc[:, t, :], ph[:, 128:136])
                nc.sync.dma_start(out=xp[:, i0:i1, :], in_=xpc[:, 0:nt, :])
                nc.scalar.dma_start(out=h1[:, i0:i1, :], in_=h1c[:, 0:nt, :])
                nc.scalar.dma_start(out=sd1[:, i0:i1, :], in_=sdc[:, 0:nt, :])
    nc.compile()
    return nc


def _edge_phase(tc, nc, cfg, aps, tail_fn, pools):
    """Shared edge phase. tail_fn(g, u_ap, pools) consumes the per-block
    post-ELU [128, 128] f32 tile."""
    T = cfg.T
    (cp, iop, gp, sp, ohp, psz_p, ps_p) = pools["cp"], pools["iop"], \
        pools["gp"], pools["sp"], pools["ohp"], pools["psz"], pools["ps"]

    iota_t = cp.tile([128, 128], TBL_DT)
    nc.sync.dma_start(out=iota_t[:], in_=aps["iota"])
    k_t = cp.tile([128, 128], F32)
    nc.sync.dma_start(out=k_t[:], in_=aps["k_rep"])
    c_t = cp.tile([128, 128], F32)
    nc.sync.dma_start(out=c_t[:], in_=aps["c_rep"])

    b0 = 0
    gq = 0
    for p, pblocks in enumerate(cfg.phases):
        tbl = aps["tbl"][p]
        idx = aps["idx"][p]
        for g0 in range(0, pblocks, cfg.gb):
            g1 = min(g0 + cfg.gb, pblocks)
            nb = g1 - g0
            n_idx = nb * T * 128
            idx_t = sp.tile([128, cfg.gb * T * 8], I16, tag="idx")
            nc.sync.dma_start(out=idx_t[:, 0:nb * T * 8],
                              in_=idx[:, g0 * T * 8:g1 * T * 8])
            v_t = gp.tile([128, cfg.gb * T, 128], TBL_DT, tag="v")
            GT = 8  # tiles per dma_gather (1024 idx, single-packet safe)
            for k0 in range(0, nb * T, GT):
                k1 = min(k0 + GT, nb * T)
                nsub = (k1 - k0) * 128
                nc.gpsimd.dma_gather(
                    out_ap=v_t[:, k0:k1, :], in_ap=tbl,
                    idxs_ap=idx_t[:, k0 * 8:k1 * 8], num_idxs=nsub,
                    num_idxs_reg=nsub, elem_size=128, single_packet=True)
            ga, gb_ = b0 + g0, b0 + g1
            sd_t = sp.tile([128, cfg.gb * T, 8], F32, tag="sd")
            nc.sync.dma_start(out=sd_t[:, 0:nb * T, :],
                              in_=aps["sd"][:, ga * T:gb_ * T, :])
            dl_t = sp.tile([128, cfg.gb * T], TBL_DT, tag="dl")
            nc.sync.dma_start(out=dl_t[:, 0:nb * T],
                              in_=aps["dst_local"][:, ga * T:gb_ * T])

            nt = nb * T
            lg_t = sp.tile([128, cfg.gb * T, 4], F32, tag="lg")
            nc.vector.tensor_tensor(
                out=lg_t[:, 0:nt, :], in0=sd_t[:, 0:nt, 0:4],
                in1=sd_t[:, 0:nt, 4:8], op=mybir.AluOpType.add)
            lr_t = sp.tile([128, cfg.gb * T, 4], F32, tag="lr")
            nc.vector.tensor_scalar(
                out=lr_t[:, 0:nt, :], in0=lg_t[:, 0:nt, :], scalar1=0.2,
                scalar2=None, op0=mybir.AluOpType.mult)
            nc.vector.tensor_tensor(
                out=lg_t[:, 0:nt, :], in0=lg_t[:, 0:nt, :],
                in1=lr_t[:, 0:nt, :], op=mybir.AluOpType.max)
            ex_t = sp.tile([128, cfg.gb * T, 4], TBL_DT, tag="ex")
            nc.scalar.activation(out=ex_t[:, 0:nt, :], in_=lg_t[:, 0:nt, :],
                                 func=mybir.ActivationFunctionType.Exp)
            ex_b = ex_t[:, 0:nt, :].unsqueeze(-1).to_broadcast(
                [128, nt, 4, 32])
            nc.vector.tensor_tensor(
                out=v_t[:, 0:nt, :].rearrange("p t (h c) -> p t h c", h=4),
                in0=v_t[:, 0:nt, :].rearrange("p t (h c) -> p t h c", h=4),
                in1=ex_b, op=mybir.AluOpType.mult)

            zs_sb = sp.tile([128, cfg.gb, 4], F32, tag="zs")
            agg_sb = gp.tile([128, cfg.gb, 128], F32, tag="aggs")
            for g in range(g0, g1):
                lt = (g - g0) * T
                psz = psz_p.tile([128, 4], F32, tag="z")
                ps = ps_p.tile([128, 128], F32, tag="agg")
                oh_blk = ohp.tile([128, T, 128], TBL_DT, tag="oh")
                dl_b = dl_t[:, lt:lt + T].unsqueeze(-1).to_broadcast(
                    [128, T, 128])
                io_b = iota_t[:].unsqueeze(1).to_broadcast([128, T, 128])
                nc.vector.tensor_tensor(out=oh_blk[:], in0=io_b, in1=dl_b,
                                        op=mybir.AluOpType.is_equal)
                for t in range(T):
                    nc.tensor.matmul(out=psz[:], lhsT=oh_blk[:, t, :],
                                     rhs=ex_t[:, lt + t, :],
                                     start=(t == 0), stop=(t == T - 1))
                    nc.tensor.matmul(out=ps[:], lhsT=oh_blk[:, t, :],
                                     rhs=v_t[:, lt + t, :],
                                     start=(t == 0), stop=(t == T - 1))
                nc.scalar.copy(zs_sb[:, g - g0, :], psz[:])
                nc.scalar.copy(agg_sb[:, g - g0, :], ps[:])
            # batched epilogue over the group's blocks
            zr_g = sp.tile([128, cfg.gb, 4], F32, tag="zrg")
            nc.vector.reciprocal(zr_g[:, 0:nb, :], zs_sb[:, 0:nb, :])
            zr_b = zr_g[:, 0:nb, :].unsqueeze(-1).to_broadcast([128, nb, 4, 32])
            u_g = gp.tile([128, cfg.gb, 128], F32, tag="ug")
            nc.vector.tensor_tensor(
                out=u_g[:, 0:nb, :].rearrange("p b (h c) -> p b h c", h=4),
                in0=agg_sb[:, 0:nb, :].rearrange("p b (h c) -> p b h c", h=4),
                in1=zr_b, op=mybir.AluOpType.mult)
            k_b = k_t[:].unsqueeze(1).to_broadcast([128, nb, 128])
            nc.vector.tensor_tensor(out=u_g[:, 0:nb, :], in0=u_g[:, 0:nb, :],
                                    in1=k_b, op=mybir.AluOpType.mult)
            c_b = c_t[:].unsqueeze(1).to_broadcast([128, nb, 128])
            nc.vector.tensor_tensor(out=u_g[:, 0:nb, :], in0=u_g[:, 0:nb, :],
                                    in1=c_b, op=mybir.AluOpType.add)
            e_g = gp.tile([128, cfg.gb, 128], F32, tag="eg")
            nc.scalar.activation(out=e_g[:, 0:nb, :], in_=u_g[:, 0:nb, :],
                                 func=mybir.ActivationFunctionType.Exp)
            nc.vector.tensor_scalar(out=e_g[:, 0:nb, :], in0=e_g[:, 0:nb, :],
                                    scalar1=-1.0, scalar2=None,
                                    op0=mybir.AluOpType.add)
            nc.vector.tensor_scalar(out=u_g[:, 0:nb, :], in0=u_g[:, 0:nb, :],
                                    scalar1=0.0, scalar2=None,
                                    op0=mybir.AluOpType.max)
            nc.vector.tensor_tensor(out=u_g[:, 0:nb, :], in0=u_g[:, 0:nb, :],
                                    in1=e_g[:, 0:nb, :], op=mybir.AluOpType.min)
            tail_fn(b0 + g0, nb, u_g, pools)
        b0 += pblocks


def _edge_inputs(nc, cfg, prefix=""):
    aps = {}
    aps["tbl"] = [nc.dram_tensor(f"tbl{p}", [cfg.phase_rows, 128], TBL_DT,
                                 kind="ExternalInput").ap()
                  for p in range(len(cfg.phases))]
    aps["idx"] = [nc.dram_tensor(
        f"idx{p}", [128, cfg.phases[p] * cfg.T * 8], I16,
        kind="ExternalInput").ap() for p in range(len(cfg.phases))]
    aps["dst_local"] = nc.dram_tensor(
        "dst_local", [128, cfg.nblk * cfg.T], TBL_DT, kind="ExternalInput").ap()
    aps["sd"] = nc.dram_tensor(
        "sd", [128, cfg.nblk * cfg.T, 8], F32, kind="ExternalInput").ap()
    aps["iota"] = nc.dram_tensor("iota", [128, 128], TBL_DT,
                                 kind="ExternalInput").ap()
    aps["k_rep"] = nc.dram_tensor("k_rep", [128, 128], F32,
                                  kind="ExternalInput").ap()
    aps["c_rep"] = nc.dram_tensor("c_rep", [128, 128], F32,
                                  kind="ExternalInput").ap()
    aps["ident"] = nc.dram_tensor("ident", [128, 128], F32,
                                  kind="ExternalInput").ap()
    return aps


def build_launch_b(cfg: Cfg):
    """Layer-1 edge phase + layer-2 node matmul."""
    nc = bacc.Bacc("TRN2", target_bir_lowering=False, debug=False,
                   num_devices=cfg.ncores)
    aps = _edge_inputs(nc, cfg)
    aps["W2cat"] = nc.dram_tensor("W2cat", [128, 136], F32,
                                  kind="ExternalInput").ap()
    h2 = nc.dram_tensor("h2", [128, cfg.nblk, 128], TBL_DT,
                        kind="ExternalOutput").ap()
    sd2 = nc.dram_tensor("sd2", [128, cfg.nblk, 8], F32,
                         kind="ExternalOutput").ap()

    with tile.TileContext(nc) as tc:
        with (
            tc.tile_pool(name="const", bufs=1) as cp,
            tc.tile_pool(name="io", bufs=2) as iop,
            tc.tile_pool(name="gat", bufs=2) as gp,
            tc.tile_pool(name="small", bufs=2) as sp,
            tc.tile_pool(name="oh", bufs=4) as ohp,
            tc.tile_pool(name="psz", bufs=2, space="PSUM") as psz_p,
            tc.tile_pool(name="ps", bufs=2, space="PSUM") as ps_p,
            tc.tile_pool(name="pst", bufs=2, space="PSUM") as pst_p,
            tc.tile_pool(name="psh", bufs=2, space="PSUM") as psh_p,
        ):
            pools = dict(cp=cp, iop=iop, gp=gp, sp=sp, ohp=ohp,
                         psz=psz_p, ps=ps_p, pst=pst_p, psh=psh_p)
            ident = cp.tile([128, 128], F32)
            nc.sync.dma_start(out=ident[:], in_=aps["ident"])
            w2 = cp.tile([128, 136], F32)
            nc.sync.dma_start(out=w2[:], in_=aps["W2cat"])

            def tail(g0_, nb_, u_g, pools):
                h2c = iop.tile([128, cfg.gb, 128], TBL_DT, tag="h2c")
                sdc = iop.tile([128, cfg.gb, 8], F32, tag="sdc")
                for i in range(nb_):
                    pt = pst_p.tile([128, 128], F32, tag="t")
                    nc.tensor.transpose(out=pt[:], in_=u_g[:, i, :],
                                        identity=ident[:])
                    o1T = gp.tile([128, 128], F32, tag="o1T")
                    nc.scalar.copy(o1T[:], pt[:])
                    ph = psh_p.tile([128, 136], F32, tag="h2")
                    nc.tensor.matmul(out=ph[:], lhsT=o1T[:], rhs=w2[:],
                                     start=True, stop=True)
                    nc.scalar.copy(h2c[:, i, :], ph[:, 0:128])
                    nc.scalar.copy(sdc[:, i, :], ph[:, 128:136])
                nc.sync.dma_start(out=h2[:, g0_:g0_ + nb_, :],
                                  in_=h2c[:, 0:nb_, :])
                nc.sync.dma_start(out=sd2[:, g0_:g0_ + nb_, :],
                                  in_=sdc[:, 0:nb_, :])

            _edge_phase(tc, nc, cfg, aps, tail, pools)
    nc.compile()
    return nc


def build_launch_c(cfg: Cfg):
    """Layer-2 edge phase + residual + MLP head + log_softmax."""
    nc = bacc.Bacc("TRN2", target_bir_lowering=False, debug=False,
                   num_devices=cfg.ncores)
    aps = _edge_inputs(nc, cfg)
    aps["xp"] = nc.dram_tensor("xp", [128, cfg.nblk, 128], F32,
                               kind="ExternalInput").ap()
    aps["Wc1f"] = nc.dram_tensor("Wc1f", [128, 64], F32,
                                 kind="ExternalInput").ap()
    aps["cc1_rep"] = nc.dram_tensor("cc1_rep", [128, 64], F32,
                                    kind="ExternalInput").ap()
    aps["Wc2"] = nc.dram_tensor("Wc2", [64, 40], F32,
                                kind="ExternalInput").ap()
    aps["bc2_rep"] = nc.dram_tensor("bc2_rep", [128, 40], F32,
                                    kind="ExternalInput").ap()
    fin = nc.dram_tensor("fin", [128, cfg.nblk, 40], F32,
                         kind="ExternalOutput").ap()

    with tile.TileContext(nc) as tc:
        with (
            tc.tile_pool(name="const", bufs=1) as cp,
            tc.tile_pool(name="io", bufs=2) as iop,
            tc.tile_pool(name="gat", bufs=2) as gp,
            tc.tile_pool(name="small", bufs=2) as sp,
            tc.tile_pool(name="oh", bufs=4) as ohp,
            tc.tile_pool(name="psz", bufs=2, space="PSUM") as psz_p,
            tc.tile_pool(name="ps", bufs=2, space="PSUM") as ps_p,
            tc.tile_pool(name="pst", bufs=2, space="PSUM") as pst_p,
            tc.tile_pool(name="psr", bufs=1, space="PSUM") as psr_p,
            tc.tile_pool(name="psy", bufs=1, space="PSUM") as psy_p,
        ):
            pools = dict(cp=cp, iop=iop, gp=gp, sp=sp, ohp=ohp,
                         psz=psz_p, ps=ps_p, pst=pst_p)
            ident = cp.tile([128, 128], F32)
            nc.sync.dma_start(out=ident[:], in_=aps["ident"])
            wc1 = cp.tile([128, 64], F32)
            nc.sync.dma_start(out=wc1[:], in_=aps["Wc1f"])
            cc1 = cp.tile([128, 64], F32)
            nc.sync.dma_start(out=cc1[:], in_=aps["cc1_rep"])
            wc2 = cp.tile([64, 40], F32)
            nc.sync.dma_start(out=wc2[:], in_=aps["Wc2"])
            bc2 = cp.tile([128, 40], F32)
            nc.sync.dma_start(out=bc2[:], in_=aps["bc2_rep"])

            def tail(g0_, nb_, u_g, pools):
                xpt = iop.tile([128, cfg.gb, 128], F32, tag="xpt")
                nc.sync.dma_start(out=xpt[:, 0:nb_, :],
                                  in_=aps["xp"][:, g0_:g0_ + nb_, :])
                nc.vector.tensor_tensor(out=u_g[:, 0:nb_, :],
                                        in0=u_g[:, 0:nb_, :],
                                        in1=xpt[:, 0:nb_, :],
                                        op=mybir.AluOpType.add)
                yc = iop.tile([128, cfg.gb, 40], F32, tag="yc")
                for i in range(nb_):
                    pt = pst_p.tile([128, 128], F32, tag="t")
                    nc.tensor.transpose(out=pt[:], in_=u_g[:, i, :],
                                        identity=ident[:])
                    o2T = gp.tile([128, 128], F32, tag="o2T")
                    nc.scalar.copy(o2T[:], pt[:])
                    pr = psr_p.tile([128, 64], F32, tag="r1")
                    nc.tensor.matmul(out=pr[:], lhsT=o2T[:], rhs=wc1[:],
                                     start=True, stop=True)
                    r1 = iop.tile([128, 64], F32, tag="r1s")
                    nc.vector.tensor_tensor(out=r1[:], in0=pr[:], in1=cc1[:],
                                            op=mybir.AluOpType.add)
                    nc.vector.tensor_scalar(out=r1[:], in0=r1[:], scalar1=0.0,
                                            scalar2=None,
                                            op0=mybir.AluOpType.max)
                    pt2 = pst_p.tile([128, 128], F32, tag="t")
                    nc.tensor.transpose(out=pt2[0:64, :], in_=r1[:],
                                        identity=ident[:])
                    r1T = iop.tile([64, 128], F32, tag="r1T")
                    nc.scalar.copy(r1T[:], pt2[0:64, :])
                    py = psy_p.tile([128, 40], F32, tag="y")
                    nc.tensor.matmul(out=py[:], lhsT=r1T[:], rhs=wc2[:],
                                     start=True, stop=True)
                    y = iop.tile([128, 40], F32, tag="y")
                    nc.vector.tensor_tensor(out=y[:], in0=py[:], in1=bc2[:],
                                            op=mybir.AluOpType.add)
                    ey = iop.tile([128, 40], F32, tag="ey")
                    nc.scalar.activation(out=ey[:], in_=y[:],
                                         func=mybir.ActivationFunctionType.Exp)
                    zs = sp.tile([128, 1], F32, tag="zss")
                    nc.vector.tensor_reduce(out=zs[:], in_=ey[:],
                                            axis=mybir.AxisListType.X,
                                            op=mybir.AluOpType.add)
                    lz = sp.tile([128, 1], F32, tag="lz")
                    nc.scalar.activation(out=lz[:], in_=zs[:],
                                         func=mybir.ActivationFunctionType.Ln)
                    nc.vector.tensor_scalar(out=yc[:, i, :], in0=y[:],
                                            scalar1=lz[:], scalar2=None,
                                            op0=mybir.AluOpType.subtract)
                nc.sync.dma_start(out=fin[:, g0_:g0_ + nb_, :],
                                  in_=yc[:, 0:nb_, :])

            _edge_phase(tc, nc, cfg, aps, tail, pools)
    nc.compile()
    return nc


# ----------------------------------------------------------------------------
# Host orchestration
# ----------------------------------------------------------------------------

_cache = {}


def _get(key, fn):
    if key not in _cache:
        _cache[key] = fn()
    return _cache[key]


def _amat(a):
    """[NH, HD] attention vector -> [128, NH] block matrix."""
    m = np.zeros((D, NH), np.float32)
    for h in range(NH):
        m[h * HD:(h + 1) * HD, h] = a[h]
    return m


def _run(nc, in_maps, cfg, tag):
    res = run_bass_kernel_spmd(nc, in_maps, list(range(cfg.ncores)),
                               trace=PROFILE)
    if PROFILE:
        LAST_EXEC_NS.append((tag, res.exec_time_ns))
    return res.results


def kernel(x, edge_index, res_W, res_b,
           W1, as1, ad1, b1, g1, be1, rm1, rv1,
           W2, as2, ad2, b2, g2, be2, rm2, rv2,
           Wc1, bc1, gc, bec, rmc, rvc, Wc2, bc2,
           _cfg=None):
    cfg = _cfg or _get("cfg", lambda: Cfg())
    x = np.asarray(x, np.float32)
    edge_index = np.asarray(edge_index)

    ekey = ("prep", hash(edge_index.tobytes()))
    prep = _get(ekey, lambda: host_prep(np.asarray(edge_index, np.int64), cfg))

    npdt = _np_dt()
    nslots_all = cfg.ncores * cfg.slots
    # node -> slot routing of x (dummies zero), transposed per core
    x_sl = np.zeros((nslots_all, IN), np.float32)
    x_sl[prep.slot_of] = x
    iota = np.tile(np.arange(128, dtype=np.float32), (128, 1)).astype(npdt)
    ident = np.eye(128, dtype=np.float32)

    def fold_bn(g_, be_, rm_, rv_, bias):
        k = (g_ / np.sqrt(rv_ + EPS_BN)).astype(np.float32)
        c = ((bias - rm_) * k + be_).astype(np.float32)
        return k, c

    k1, c1 = fold_bn(g1, be1, rm1, rv1, b1)
    k2, c2 = fold_bn(g2, be2, rm2, rv2, b2)
    kc, cc = fold_bn(gc, bec, rmc, rvc, bc1)
    rep = lambda v: np.tile(np.asarray(v, np.float32), (128, 1))

    W1cat = np.concatenate(
        [W1, W1 @ _amat(as1), W1 @ _amat(ad1)], axis=1).astype(np.float32)
    W2cat = np.concatenate(
        [W2, W2 @ _amat(as2), W2 @ _amat(ad2)], axis=1).astype(np.float32)
    Wc1f = (Wc1 * kc[None, :]).astype(np.float32)

    # ---- launch A ----
    nc_a = _get(("A", cfg.T), lambda: build_launch_a(cfg))
    in_a = []
    for c in range(cfg.ncores):
        xs = x_sl[c * cfg.slots:(c + 1) * cfg.slots]
        in_a.append(dict(
            xT=np.ascontiguousarray(xs.T), res_W=np.asarray(res_W, np.float32),
            res_b_col=np.asarray(res_b, np.float32).reshape(128, 1),
            res_b_rep=rep(res_b), W1cat=W1cat))
    res_a = _run(nc_a, in_a, cfg, "A")

    # h/s/d in slot order ([128, nblk, c] -> [slots, c])
    def slotify(arr, cdim):
        return arr.transpose(1, 0, 2).reshape(cfg.slots, cdim)

    h1_full = np.concatenate(
        [slotify(res_a[c]["h1"], 128) for c in range(cfg.ncores)])
    sd1_full = np.concatenate(
        [slotify(res_a[c]["sd1"], 8) for c in range(cfg.ncores)])
    xp_dev = [res_a[c]["xp"] for c in range(cfg.ncores)]

    # ---- launch B ----
    tabs1 = build_tables(prep, cfg, h1_full.astype(npdt))
    sd_e1 = expand_sd(prep, cfg, sd1_full[:, 0:4], sd1_full[:, 4:8])
    nc_b = _get(("B", cfg.T, cfg.phase_rows), lambda: build_launch_b(cfg))
    in_b = []
    for c in range(cfg.ncores):
        pc = prep.per_core[c]
        m = dict(dst_local=pc.dst_local.astype(npdt),
                 sd=sd_e1[c], iota=iota, ident=ident,
                 k_rep=rep(k1), c_rep=rep(c1), W2cat=W2cat)
        for p in range(len(cfg.phases)):
            m[f"tbl{p}"] = tabs1[c][p]
            m[f"idx{p}"] = pc.idx16[p]
        in_b.append(m)
    res_b_ = _run(nc_b, in_b, cfg, "B")

    h2_full = np.concatenate(
        [slotify(res_b_[c]["h2"], 128) for c in range(cfg.ncores)])
    sd2_full = np.concatenate(
        [slotify(res_b_[c]["sd2"], 8) for c in range(cfg.ncores)])

    # ---- launch C ----
    tabs2 = build_tables(prep, cfg, h2_full.astype(npdt))
    sd_e2 = expand_sd(prep, cfg, sd2_full[:, 0:4], sd2_full[:, 4:8])
    nc_c = _get(("C", cfg.T, cfg.phase_rows), lambda: build_launch_c(cfg))
    in_c = []
    for c in range(cfg.ncores):
        pc = prep.per_core[c]
        m = dict(dst_local=pc.dst_local.astype(npdt),
                 sd=sd_e2[c], iota=iota, ident=ident,
                 k_rep=rep(k2), c_rep=rep(c2), xp=xp_dev[c],
                 Wc1f=Wc1f, cc1_rep=rep(cc), Wc2=np.asarray(Wc2, np.float32),
                 bc2_rep=rep(bc2))
        for p in range(len(cfg.phases)):
            m[f"tbl{p}"] = tabs2[c][p]
            m[f"idx{p}"] = pc.idx16[p]
        in_c.append(m)
    res_c = _run(nc_c, in_c, cfg, "C")

    fin_slots = np.concatenate(
        [slotify(res_c[c]["fin"], 40) for c in range(cfg.ncores)])
    return np.ascontiguousarray(fin_slots[prep.slot_of]).astype(np.float32)

